# revision 36
# baseline (speedup 1.0000x reference)
"""Trainium2 Bass kernel for BasicSelfAttention2D (spatial-reduction attention).

Reference computation (per image):
    q   = (wq @ x_flat)              [d=32, N=4096]
    xkv = avgpool2x2(x)              [C, Nk=1024]
    k   = wk @ xkv                   [d, Nk]
    v   = wv @ xkv                   [C, Nk]
    attn= softmax(q^T k / sqrt(d))   [N, Nk]
    out = v @ attn^T                 [C, N]
    y   = x + gamma * (wo @ out)

Sharding: data-parallel over batch, one image per NeuronCore (8 cores).

Kernel design:
  - The residual is computed from the bf16 copy of x already in SBUF; the
    fp32 x is never loaded (halves input HBM traffic; rel-err budget 2e-2,
    achieved ~3.7e-3).
  - q/k projections use host-stacked 4x-replicated weights, so the
    projection matmuls directly produce q/k replicated across all four
    32-partition bands at full 128-col PE utilization.
  - Scores are built TRANSPOSED s_T[m, n] in 2-way row-packed "packs"
    (K=32 matmuls via tile_position; consecutive packs alternate band
    pairs so four score matmuls overlap in the array).  One 1024-wide exp
    (softmax scale folded in) evacuates each pack into fp8e4m3 directly in
    the DoubleRow-paired layout et[k, j, n].  Packs double-buffer through
    2 PSUM tiles so exp(q) overlaps the matmuls of pack q+1.
  - Attention aggregation, softmax row-sums (ones weights, pair stride
    16), and the out-projection run as fp8 DoubleRow matmuls - 2
    contraction rows/cycle, halving PE time on ~75% of the FLOPs.
  - Softmax denominator: row-sums via DR ones-matmuls into [1,512] PSUM,
    f16 row copy, K=1 broadcast matmul, reciprocal_approx_fast; applied
    as a per-column scale in the PSUM->SBUF evacuation of the aggregation
    output (it commutes with the out-projection).
  - Pipelining: everything is emitted so no PE instruction waits on work
    of its own super (a PE stall >3.4us re-throttles the HAM clock gate
    to 1.2 GHz, doubling every matmul):
      * score packs + exps for super s+1 are spread through super s;
      * row-sum chains for super s+1 run during super s, with their g2/g3
        tails sliding into super s+1;
      * the reciprocal broadcast runs at super start from the previous
        iteration's row-sums;
      * aggregation chains run first, against et tiles exp'd a super ago.
  - The head is xb-DMA-paced: as each 1024-col chunk lands, its q-proj,
    avgpool (split DVE / GpSimd), 256-wide k-proj chunk, super-0 score
    packs and v-projections are issued.  Weights arrive as two contiguous
    DMAs - [wq|wk] first (copied early on DVE, unblocking q/k matmuls),
    [wv|wo] behind the first xb chunk (copied on ACT); 5 dummy matmuls
    warm the clock gate during the DMA.
  - PSUM budget: score packs 2x2 + agg/proj 2 + rowsum/broadcast 2 = 8
    banks exactly.
  - host-side prep folds: 1/4 (avgpool mean) into wkT/wvT, gamma into
    woT, softmax scale into the exp activation.
"""

import ml_dtypes
import numpy as np

import concourse.bacc as bacc
import concourse.mybir as mybir
from concourse.tile import TileContext
from concourse.bass_utils import run_bass_kernel_spmd

B, C, H, W = 8, 256, 64, 64
N = H * W          # 4096
D = 32             # q/k dim
NK = (H // 2) * (W // 2)   # 1024
NCORES = 8

F32 = mybir.dt.float32
F16 = mybir.dt.float16
BF16 = mybir.dt.bfloat16
F8 = mybir.dt.float8e4

SCALE = 1.0 / np.sqrt(np.float32(D))   # softmax scale

SUP = 1024          # n-super width
NSUP = N // SUP     # 4
NCHUNK = 512        # matmul free-dim chunk
MT = NK // 128      # 8 m-tiles
NG = MT // 2        # 4 kv chain-pairs (DoubleRow contracts 256 at a time)

DR = mybir.MatmulPerfMode.DoubleRow
EXP = mybir.ActivationFunctionType.Exp
COPY = mybir.ActivationFunctionType.Copy


def build_nc():
    nc = bacc.Bacc(None, target_bir_lowering=False, debug=False)

    xb_in = nc.dram_tensor("xb", [C, N], BF16, kind="ExternalInput")
    WPACK = 128 + 128 + C + C   # wq4 | wk4 | wvT | woT along the free dim
    wqk_in = nc.dram_tensor("wqk", [C, 256], BF16, kind="ExternalInput")
    wvo_in = nc.dram_tensor("wvo", [C, 2 * C], BF16, kind="ExternalInput")
    y_out = nc.dram_tensor("y", [C, N], F32, kind="ExternalOutput")

    with TileContext(nc) as tc:
        with (
            tc.tile_pool(name="big", bufs=1) as big,
            tc.tile_pool(name="scl", bufs=4) as sclp,
            tc.tile_pool(name="rows", bufs=4) as rowp,
            tc.tile_pool(name="outu", bufs=2) as outup,
            tc.tile_pool(name="ystage", bufs=4) as ypool,
            tc.tile_pool(name="ps_sc", bufs=2, space="PSUM") as ps_sc,
            tc.tile_pool(name="ps_av", bufs=2, space="PSUM") as ps_av,
            tc.tile_pool(name="ps_rs", bufs=2, space="PSUM") as ps_rs,
        ):
            # ---------------- persistent SBUF ----------------
            xb_sb = big.tile([128, 2, N], BF16, tag="xb")     # c-half major
            xkv_sb = big.tile([128, 2, NK], BF16, tag="xkv")
            qrep_sb = big.tile([128, N], BF16, tag="qrep")    # q replicated 4x
            krep_sb = big.tile([128, NK], BF16, tag="krep")   # k replicated 4x
            # exp(scores) fp8, double-buffered across supers: [k, s%2, g, j, n]
            etbig = big.tile([128, 2, NG, 2, SUP], F8, tag="etbig")
            # v transposed, fp8, paired for DoubleRow: [k, g, j, c]
            vT4_sb = big.tile([128, NG, 2, C], F8, tag="vT4")
            # wo fp8 pairs: [k, j, ot, oc]
            wo8_sb = big.tile([128, 2, 2, 128], F8, tag="wo8")
            # DMA staging for weights; the ACT copy into the real tile makes
            # every matmul weight-dependency an ACT-sem wait.
            w_st = big.tile([128, 2, WPACK], BF16, tag="w_st")
            w_sb = big.tile([128, 2, WPACK], BF16, tag="w_sb")
            wq_sb = w_sb[:, :, 0:128]
            wk_sb = w_sb[:, :, 128:256]
            wv_sb = w_sb[:, :, 256 : 256 + C]
            wo_sb = w_sb[:, :, 256 + C :]

            # ---------------- input DMAs ----------------
            # wq first (small) so q-proj unblocks as soon as xb chunk 0
            # lands; the remaining weights stream in behind the first chunk.
            nc.sync.dma_start(
                out=w_st[:, :, 0:256],
                in_=wqk_in.rearrange("(t p) w -> p t w", p=128),
            )
            nsl0 = slice(0, SUP)
            nc.sync.dma_start(out=xb_sb[:, 0, nsl0], in_=xb_in[0:128, nsl0])
            nc.sync.dma_start(out=xb_sb[:, 1, nsl0], in_=xb_in[128:256, nsl0])
            nc.sync.dma_start(
                out=w_st[:, :, 256:],
                in_=wvo_in.rearrange("(t p) w -> p t w", p=128),
            )
            for s in range(1, NSUP):
                nsl = slice(s * SUP, (s + 1) * SUP)
                for ch in range(2):
                    rows = slice(ch * 128, (ch + 1) * 128)
                    nc.sync.dma_start(out=xb_sb[:, ch, nsl], in_=xb_in[rows, nsl])
            nc.scalar.activation(out=w_sb[:, :, 0:256], in_=w_st[:, :, 0:256],
                                 func=COPY)
            nc.scalar.activation(out=w_sb[:, :, 256:], in_=w_st[:, :, 256:],
                                 func=COPY)
            # fp8 copies of wo (paired) and the DR ones weights
            nc.scalar.activation(out=wo8_sb.rearrange("p a b c -> p a (b c)"),
                                 in_=wo_sb, func=COPY)
            ones8 = big.tile([128, 2, 16], F8, tag="ones8")
            nc.vector.memset(ones8, 1.0)
            ones_row = big.tile([1, 128], F16, tag="ones_row")
            nc.vector.memset(ones_row, 1.0)
            # dummy exp: pulls the ACT exp table load into the setup phase
            warm = big.tile([128, 1], F32, tag="warm")
            nc.vector.memset(warm, 0.0)
            nc.scalar.activation(out=warm, in_=warm, func=EXP)
            # HAM warm-up: dummy matmuls fill the PE-idle DMA-wait window
            wrm_sb = big.tile([128, NCHUNK], BF16, tag="wrm")
            nc.vector.memset(wrm_sb, 0.0)
            wrm_ps = ps_av.tile([128, NCHUNK], F32, tag="av", name="wrm_ps")
            for i in range(5):
                nc.tensor.matmul(
                    wrm_ps, lhsT=wrm_sb[:, 0:128], rhs=wrm_sb,
                    start=(i == 0), stop=(i == 4),
                )

            # ---------------- projections, pooling, score fill ------------
            # per-chunk pipelining against the xb DMA: q-proj chunk as it
            # lands, pooling split across DVE (c-half 0) and GpSimd (c-half
            # 1), k-proj per m-half, score quads + v-proj interleaved.
            def qproj(cn):
                nsl = slice(cn * NCHUNK, (cn + 1) * NCHUNK)
                qp = ps_av.tile([128, NCHUNK], F32, tag="av", name="qp")
                for ch in range(2):
                    nc.tensor.matmul(
                        qp, lhsT=wq_sb[:, ch, :], rhs=xb_sb[:, ch, nsl],
                        start=(ch == 0), stop=(ch == 1),
                    )
                nc.vector.tensor_copy(out=qrep_sb[:, nsl], in_=qp)

            xw = big.tile([128, 2, 64, 32], BF16, tag="xw")

            def pool(s):
                for ch in range(2):
                    eng = nc.vector if ch == 0 else nc.gpsimd
                    x4 = xb_sb[:, ch, :].rearrange(
                        "p (h w t) -> p h w t", h=64, w=32)
                    hs = slice(s * 16, (s + 1) * 16)
                    eng.tensor_add(
                        out=xw[:, ch, hs, :], in0=x4[:, hs, :, 0],
                        in1=x4[:, hs, :, 1],
                    )
                    xh = xw[:, ch].rearrange("p (h2 t) w -> p h2 t w", t=2)
                    xkv_v = xkv_sb[:, ch, :].rearrange("p (a b) -> p a b", a=32)
                    h2s = slice(s * 8, (s + 1) * 8)
                    eng.tensor_add(
                        out=xkv_v[:, h2s, :],
                        in0=xh[:, h2s, 0, :],
                        in1=xh[:, h2s, 1, :],
                    )

            def kproj(cn):
                # per-256 m-chunk so score packs can chase the xb DMA
                nsl = slice(cn * 256, (cn + 1) * 256)
                kp = ps_av.tile([128, 256], F32, tag="av", name="kp")
                for ch in range(2):
                    nc.tensor.matmul(
                        kp, lhsT=wk_sb[:, ch, :], rhs=xkv_sb[:, ch, nsl],
                        start=(ch == 0), stop=(ch == 1),
                    )
                nc.vector.tensor_copy(out=krep_sb[:, nsl], in_=kp)

            # ---------------- helpers ----------------
            def vproj(mt):
                msl = slice(mt * 128, (mt + 1) * 128)
                vp = ps_av.tile([128, C], F32, tag="av", name="vp")
                for ch in range(2):
                    nc.tensor.matmul(
                        vp, lhsT=xkv_sb[:, ch, msl], rhs=wv_sb[:, ch, :],
                        start=(ch == 0), stop=(ch == 1),
                    )
                nc.scalar.activation(
                    out=vT4_sb[:, mt // 2, mt % 2, :], in_=vp, func=COPY
                )

            def quad(s, p, h):
                """2-way row-packed score pack: kv pair p (mts 2p, 2p+1),
                n-half h of super s; one 1024-wide exp into the paired fp8
                layout.  Packs double-buffer through ps_sc so exp(q)
                overlaps the score matmuls of pack q+1; consecutive packs
                alternate row-band pairs so their matmuls can overlap."""
                et = etbig[:, s % 2]
                sc_ps = ps_sc.tile([128, 2, NCHUNK], F32, tag="sc", name="scq")
                hsl = slice(s * SUP + h * NCHUNK, s * SUP + (h + 1) * NCHUNK)
                bb = 2 * ((2 * p + h) % 2)   # band pair alternation
                for i in range(2):
                    mt = 2 * p + i
                    band = slice(32 * (bb + i), 32 * (bb + i + 1))
                    nc.tensor.matmul(
                        sc_ps[:, i, :],
                        lhsT=krep_sb[band, mt * 128 : (mt + 1) * 128],
                        rhs=qrep_sb[band, hsl],
                        tile_position=(32 * (bb + i), 0),
                    )
                osl = slice(h * NCHUNK, (h + 1) * NCHUNK)
                nc.scalar.activation(
                    out=et[:, p, :, osl],
                    in_=sc_ps, func=EXP, scale=float(SCALE),
                )

            def make_rs(s):
                """Row-sum state for super s: two DR ones-matmul chains (one
                per n-half) over the 4 kv pairs, emitted in two stages."""
                et = etbig[:, s % 2]
                rs_ps = [
                    ps_rs.tile([1, NCHUNK], F32, tag="rs", name=f"rs{s}_{h}")
                    for h in range(2)
                ]

                def rs_part(h, gs):
                    osl = slice(h * NCHUNK, (h + 1) * NCHUNK)
                    for g in gs:
                        nc.tensor.matmul(
                            rs_ps[h], lhsT=ones8[:, :, 0:1],
                            rhs=et[:, g, :, osl],
                            start=(g == 0), stop=(g == NG - 1), perf_mode=DR,
                        )
                    if gs[-1] == NG - 1:
                        rr = rowp.tile([1, NCHUNK], F16, tag="rs_row")
                        nc.vector.tensor_copy(out=rr, in_=rs_ps[h])
                        return rr

                return rs_part

            # ---------------- pipeline fill (super 0 head) ----------------
            # DMA-paced: after xb chunk cs lands -> q-proj, pooling, k-proj
            # (m quarter cs), super-0 score packs for kv-pair cs, v-proj.
            rs_cur = make_rs(0)
            for cs in range(4):
                qproj(2 * cs); qproj(2 * cs + 1)
                pool(cs)
                kproj(cs)
                quad(0, cs, 0); quad(0, cs, 1)
                vproj(2 * cs); vproj(2 * cs + 1)
                if cs == 2:
                    rs_cur(0, [0, 1])
                if cs == 3:
                    rs_cur(1, [0, 1])

            # ---------------- main loop over n-supers ----------------
            for s in range(NSUP):
                et = etbig[:, s % 2]
                rs_here = rs_cur

                def nquad(p, h):
                    if s + 1 < NSUP:
                        quad(s + 1, p, h)

                outu4 = outup.tile([128, 2, SUP], F8, tag="outu")
                scale_sb = {}

                def agg_chains(c, gs):
                    if gs[0] == 0:
                        agg_ps[c] = {
                            h: ps_av.tile([128, NCHUNK], F32, tag="av",
                                          name=f"av{c}{h}")
                            for h in range(2)
                        }
                    for g in gs:
                        for h in range(2):
                            osl = slice(h * NCHUNK, (h + 1) * NCHUNK)
                            nc.tensor.matmul(
                                agg_ps[c][h],
                                lhsT=vT4_sb[:, g, :, c * 128 : (c + 1) * 128],
                                rhs=et[:, g, :, osl],
                                start=(g == 0), stop=(g == NG - 1),
                                perf_mode=DR,
                            )

                def stt(c):
                    for h in range(2):
                        osl = slice(h * NCHUNK, (h + 1) * NCHUNK)
                        nc.vector.scalar_tensor_tensor(
                            out=outu4[:, c, osl],
                            in0=agg_ps[c][h],
                            scalar=1.0,
                            in1=scale_sb[h],
                            op0=mybir.AluOpType.mult,
                            op1=mybir.AluOpType.mult,
                        )

                agg_ps = {}
                # 1. aggregation c=0; first next-super quads interleave
                agg_chains(0, [0, 1])
                nquad(0, 0); nquad(0, 1)
                if s + 1 >= NSUP:
                    # no next-super packs to cover the row-copy latency:
                    # let the second half of agg c0 do it instead
                    rows = [rs_here(0, [2, 3]), rs_here(1, [2, 3])]
                    agg_chains(0, [2, 3])
                else:
                    agg_chains(0, [2, 3])
                    nquad(1, 0); nquad(1, 1)
                    rows = [rs_here(0, [2, 3]), rs_here(1, [2, 3])]
                # 4. reciprocal broadcast -> per-column scales
                for h in range(2):
                    bc_ps = ps_rs.tile([128, NCHUNK], F32, tag="rs", name="bc")
                    nc.tensor.matmul(bc_ps, lhsT=ones_row, rhs=rows[h])
                    sc_t = sclp.tile([128, NCHUNK], F32, tag="scale")
                    nc.vector.reciprocal_approx_fast(out=sc_t, in_=bc_ps)
                    scale_sb[h] = sc_t
                # rs tiles for s+1 alloc AFTER the bc tiles (ps_rs rotation)
                rs_nxt = make_rs(s + 1) if s + 1 < NSUP else None
                # 5. evacuate c=0; aggregate c=1 with quads interleaved
                stt(0)
                agg_chains(1, [0, 1])
                nquad(2, 0); nquad(2, 1)
                agg_chains(1, [2, 3])
                stt(1)
                if rs_nxt is not None:
                    rs_nxt(0, [0, 1])
                nquad(3, 0); nquad(3, 1)

                # 6. out-projection (fp8 DR) + residual add + store
                for ot in range(2):
                    for half in range(2):
                        osl = slice(half * NCHUNK, (half + 1) * NCHUNK)
                        fsl = slice(s * SUP + half * NCHUNK,
                                    s * SUP + (half + 1) * NCHUNK)
                        op_ps = ps_av.tile([128, NCHUNK], F32, tag="av",
                                           name="op")
                        nc.tensor.matmul(
                            op_ps, lhsT=wo8_sb[:, :, ot, :],
                            rhs=outu4[:, :, osl], perf_mode=DR,
                        )
                        y_st = ypool.tile([128, NCHUNK], F32, tag="y")
                        nc.vector.tensor_add(
                            out=y_st, in0=op_ps, in1=xb_sb[:, ot, fsl]
                        )
                        nc.sync.dma_start(
                            out=y_out[ot * 128 : (ot + 1) * 128, fsl], in_=y_st
                        )
                    if ot == 0:
                        if rs_nxt is not None:
                            rs_nxt(1, [0, 1])
                rs_cur = rs_nxt
    nc.compile()
    return nc


_NC_CACHE = {}


def _get_nc():
    if "nc" not in _NC_CACHE:
        _NC_CACHE["nc"] = build_nc()
    return _NC_CACHE["nc"]


def _prep_inputs(x, wq, wk, wv, wo, gamma):
    bf = ml_dtypes.bfloat16
    x = np.asarray(x, dtype=np.float32)
    xb = x.astype(bf)
    wq4 = np.tile(np.asarray(wq, np.float32).T, (1, 4))
    wk4 = np.tile(np.asarray(wk, np.float32).T * 0.25, (1, 4))
    wvT = np.asarray(wv, np.float32).T * 0.25
    woT = np.float32(np.asarray(gamma, np.float32)[0]) * np.asarray(
        wo, np.float32
    ).T
    wqk = np.ascontiguousarray(
        np.concatenate([wq4, wk4], axis=1)).astype(bf)
    wvo = np.ascontiguousarray(
        np.concatenate([wvT, woT], axis=1)).astype(bf)
    in_maps = []
    for i in range(NCORES):
        in_maps.append({
            "xb": np.ascontiguousarray(xb[i].reshape(C, N)),
            "wqk": wqk,
            "wvo": wvo,
        })
    return in_maps


def run(x, wq, wk, wv, wo, gamma, trace=False, **trace_kwargs):
    nc = _get_nc()
    in_maps = _prep_inputs(x, wq, wk, wv, wo, gamma)
    res = run_bass_kernel_spmd(
        nc, in_maps, list(range(NCORES)), trace=trace, **trace_kwargs
    )
    y = np.stack([res.results[i]["y"].reshape(C, H, W) for i in range(NCORES)])
    return y, res


def kernel(x, wq, wk, wv, wo, gamma):
    y, _ = run(x, wq, wk, wv, wo, gamma, trace=False)
    return y


# revision 37
# speedup vs baseline: 1.1653x; 1.1653x over previous
"""Trainium2 Bass kernel for BasicSelfAttention2D (spatial-reduction attention).

Reference computation (per image):
    q   = (wq @ x_flat)              [d=32, N=4096]
    xkv = avgpool2x2(x)              [C, Nk=1024]
    k   = wk @ xkv                   [d, Nk]
    v   = wv @ xkv                   [C, Nk]
    attn= softmax(q^T k / sqrt(d))   [N, Nk]
    out = v @ attn^T                 [C, N]
    y   = x + gamma * (wo @ out)

Sharding: data-parallel over batch, one image per NeuronCore (8 cores).

Kernel design:
  - The residual is computed from the bf16 copy of x already in SBUF; the
    fp32 x is never loaded (halves input HBM traffic; rel-err budget 2e-2,
    achieved ~3.7e-3).
  - q/k projections use host-stacked 4x-replicated weights, so the
    projection matmuls directly produce q/k replicated across all four
    32-partition bands at full 128-col PE utilization.
  - Scores are built TRANSPOSED s_T[m, n] in 2-way row-packed "packs"
    (K=32 matmuls via tile_position; consecutive packs alternate band
    pairs so four score matmuls overlap in the array).  One 1024-wide exp
    (softmax scale folded in) evacuates each pack into fp8e4m3 directly in
    the DoubleRow-paired layout et[k, j, n].  Packs double-buffer through
    2 PSUM tiles so exp(q) overlaps the matmuls of pack q+1.
  - Attention aggregation, softmax row-sums (ones weights, pair stride
    16), and the out-projection run as fp8 DoubleRow matmuls - 2
    contraction rows/cycle, halving PE time on ~75% of the FLOPs.
  - Softmax denominator: row-sums via DR ones-matmuls into [1,512] PSUM,
    f16 row copy, K=1 broadcast matmul, reciprocal_approx_fast; applied
    as a per-column scale in the PSUM->SBUF evacuation of the aggregation
    output (it commutes with the out-projection).
  - Pipelining: everything is emitted so no PE instruction waits on work
    of its own super (a PE stall >3.4us re-throttles the HAM clock gate
    to 1.2 GHz, doubling every matmul):
      * score packs + exps for super s+1 are spread through super s;
      * row-sum chains for super s+1 run during super s, with their g2/g3
        tails sliding into super s+1;
      * the reciprocal broadcast runs at super start from the previous
        iteration's row-sums;
      * aggregation chains run first, against et tiles exp'd a super ago.
  - The head is xb-DMA-paced: as each 1024-col chunk lands, its q-proj,
    avgpool (split DVE / GpSimd), 256-wide k-proj chunk, super-0 score
    packs and v-projections are issued.  Weights arrive as two contiguous
    DMAs - [wq|wk] first (copied early on DVE, unblocking q/k matmuls),
    [wv|wo] behind the first xb chunk (copied on ACT); 5 dummy matmuls
    warm the clock gate during the DMA.
  - PSUM budget: score packs 2x2 + agg/proj 2 + rowsum/broadcast 2 = 8
    banks exactly.
  - host-side prep folds: 1/4 (avgpool mean) into wkT/wvT, gamma into
    woT, softmax scale into the exp activation.
"""

import ml_dtypes
import numpy as np

import concourse.bacc as bacc
import concourse.mybir as mybir
from concourse.tile import TileContext
from concourse.bass_utils import run_bass_kernel_spmd

B, C, H, W = 8, 256, 64, 64
N = H * W          # 4096
D = 32             # q/k dim
NK = (H // 2) * (W // 2)   # 1024
NCORES = 8

F32 = mybir.dt.float32
F16 = mybir.dt.float16
BF16 = mybir.dt.bfloat16
F8 = mybir.dt.float8e4

SCALE = 1.0 / np.sqrt(np.float32(D))   # softmax scale

SUP = 1024          # n-super width
NSUP = N // SUP     # 4
NCHUNK = 512        # matmul free-dim chunk
MT = NK // 128      # 8 m-tiles
NG = MT // 2        # 4 kv chain-pairs (DoubleRow contracts 256 at a time)

DR = mybir.MatmulPerfMode.DoubleRow
EXP = mybir.ActivationFunctionType.Exp
COPY = mybir.ActivationFunctionType.Copy


def build_nc():
    nc = bacc.Bacc(None, target_bir_lowering=False, debug=False)

    xb_in = nc.dram_tensor("xb", [C, N], BF16, kind="ExternalInput")
    WPACK = 128 + 128 + C + C   # wq4 | wk4 | wvT | woT along the free dim
    wqk_in = nc.dram_tensor("wqk", [C, 256], BF16, kind="ExternalInput")
    wvo_in = nc.dram_tensor("wvo", [C, 2 * C], BF16, kind="ExternalInput")
    y_out = nc.dram_tensor("y", [C, N], F32, kind="ExternalOutput")

    with TileContext(nc) as tc:
        with (
            tc.tile_pool(name="big", bufs=1) as big,
            tc.tile_pool(name="scl", bufs=4) as sclp,
            tc.tile_pool(name="rows", bufs=4) as rowp,
            tc.tile_pool(name="outu", bufs=2) as outup,
            tc.tile_pool(name="ystage", bufs=4) as ypool,
            tc.tile_pool(name="ps_sc", bufs=2, space="PSUM") as ps_sc,
            tc.tile_pool(name="ps_av", bufs=2, space="PSUM") as ps_av,
            tc.tile_pool(name="ps_rs", bufs=2, space="PSUM") as ps_rs,
        ):
            # ---------------- persistent SBUF ----------------
            xb_sb = big.tile([128, 2, N], BF16, tag="xb")     # c-half major
            xkv_sb = big.tile([128, 2, NK], BF16, tag="xkv")
            qrep_sb = big.tile([128, N], BF16, tag="qrep")    # q replicated 4x
            krep_sb = big.tile([128, NK], BF16, tag="krep")   # k replicated 4x
            # exp(scores) fp8, double-buffered across supers: [k, s%2, g, j, n]
            etbig = big.tile([128, 2, NG, 2, SUP], F8, tag="etbig")
            # v transposed, fp8, paired for DoubleRow: [k, g, j, c]
            vT4_sb = big.tile([128, NG, 2, C], F8, tag="vT4")
            # wo fp8 pairs: [k, j, ot, oc]
            wo8_sb = big.tile([128, 2, 2, 128], F8, tag="wo8")
            # DMA staging for weights; the ACT copy into the real tile makes
            # every matmul weight-dependency an ACT-sem wait.
            w_st = big.tile([128, 2, WPACK], BF16, tag="w_st")
            w_sb = big.tile([128, 2, WPACK], BF16, tag="w_sb")
            wq_sb = w_sb[:, :, 0:128]
            wk_sb = w_sb[:, :, 128:256]
            wv_sb = w_sb[:, :, 256 : 256 + C]
            wo_sb = w_sb[:, :, 256 + C :]

            # ---------------- input DMAs ----------------
            # wq first (small) so q-proj unblocks as soon as xb chunk 0
            # lands; the remaining weights stream in behind the first chunk.
            nc.sync.dma_start(
                out=w_st[:, :, 0:256],
                in_=wqk_in.rearrange("(t p) w -> p t w", p=128),
            )
            nsl0 = slice(0, SUP)
            nc.sync.dma_start(out=xb_sb[:, 0, nsl0], in_=xb_in[0:128, nsl0])
            nc.sync.dma_start(out=xb_sb[:, 1, nsl0], in_=xb_in[128:256, nsl0])
            nc.sync.dma_start(
                out=w_st[:, :, 256:],
                in_=wvo_in.rearrange("(t p) w -> p t w", p=128),
            )
            for s in range(1, NSUP):
                nsl = slice(s * SUP, (s + 1) * SUP)
                for ch in range(2):
                    rows = slice(ch * 128, (ch + 1) * 128)
                    nc.sync.dma_start(out=xb_sb[:, ch, nsl], in_=xb_in[rows, nsl])
            nc.scalar.activation(out=w_sb[:, :, 0:256], in_=w_st[:, :, 0:256],
                                 func=COPY)
            nc.scalar.activation(out=w_sb[:, :, 256:], in_=w_st[:, :, 256:],
                                 func=COPY)
            # fp8 copies of wo (paired) and the DR ones weights
            nc.scalar.activation(out=wo8_sb.rearrange("p a b c -> p a (b c)"),
                                 in_=wo_sb, func=COPY)
            ones8 = big.tile([128, 2, 16], F8, tag="ones8")
            nc.vector.memset(ones8, 1.0)
            ones_row = big.tile([1, 128], F16, tag="ones_row")
            nc.vector.memset(ones_row, 1.0)
            # dummy exp: pulls the ACT exp table load into the setup phase
            warm = big.tile([128, 1], F32, tag="warm")
            nc.vector.memset(warm, 0.0)
            nc.scalar.activation(out=warm, in_=warm, func=EXP)
            # HAM warm-up: dummy matmuls fill the PE-idle DMA-wait window
            wrm_sb = big.tile([128, NCHUNK], BF16, tag="wrm")
            nc.vector.memset(wrm_sb, 0.0)
            wrm_ps = ps_av.tile([128, NCHUNK], F32, tag="av", name="wrm_ps")
            # 10 cold matmuls = 4.3us of sustained PE busy: guarantees one
            # full HAM SHORT window fires so the head runs at 2.4 GHz
            for i in range(10):
                nc.tensor.matmul(
                    wrm_ps, lhsT=wrm_sb[:, 0:128], rhs=wrm_sb,
                    start=(i == 0), stop=(i == 9),
                )

            # ---------------- projections, pooling, score fill ------------
            # per-chunk pipelining against the xb DMA: q-proj chunk as it
            # lands, pooling split across DVE (c-half 0) and GpSimd (c-half
            # 1), k-proj per m-half, score quads + v-proj interleaved.
            def qproj(cn):
                nsl = slice(cn * NCHUNK, (cn + 1) * NCHUNK)
                qp = ps_av.tile([128, NCHUNK], F32, tag="av", name="qp")
                for ch in range(2):
                    nc.tensor.matmul(
                        qp, lhsT=wq_sb[:, ch, :], rhs=xb_sb[:, ch, nsl],
                        start=(ch == 0), stop=(ch == 1),
                    )
                nc.vector.tensor_copy(out=qrep_sb[:, nsl], in_=qp)

            xw = big.tile([128, 2, 64, 32], BF16, tag="xw")

            def pool(s):
                for ch in range(2):
                    eng = nc.vector if ch == 0 else nc.gpsimd
                    x4 = xb_sb[:, ch, :].rearrange(
                        "p (h w t) -> p h w t", h=64, w=32)
                    hs = slice(s * 16, (s + 1) * 16)
                    eng.tensor_add(
                        out=xw[:, ch, hs, :], in0=x4[:, hs, :, 0],
                        in1=x4[:, hs, :, 1],
                    )
                    xh = xw[:, ch].rearrange("p (h2 t) w -> p h2 t w", t=2)
                    xkv_v = xkv_sb[:, ch, :].rearrange("p (a b) -> p a b", a=32)
                    h2s = slice(s * 8, (s + 1) * 8)
                    eng.tensor_add(
                        out=xkv_v[:, h2s, :],
                        in0=xh[:, h2s, 0, :],
                        in1=xh[:, h2s, 1, :],
                    )

            def kproj(cn):
                # per-256 m-chunk so score packs can chase the xb DMA
                nsl = slice(cn * 256, (cn + 1) * 256)
                kp = ps_av.tile([128, 256], F32, tag="av", name="kp")
                for ch in range(2):
                    nc.tensor.matmul(
                        kp, lhsT=wk_sb[:, ch, :], rhs=xkv_sb[:, ch, nsl],
                        start=(ch == 0), stop=(ch == 1),
                    )
                nc.vector.tensor_copy(out=krep_sb[:, nsl], in_=kp)

            # ---------------- helpers ----------------
            def vproj(mt):
                msl = slice(mt * 128, (mt + 1) * 128)
                vp = ps_av.tile([128, C], F32, tag="av", name="vp")
                for ch in range(2):
                    nc.tensor.matmul(
                        vp, lhsT=xkv_sb[:, ch, msl], rhs=wv_sb[:, ch, :],
                        start=(ch == 0), stop=(ch == 1),
                    )
                nc.scalar.activation(
                    out=vT4_sb[:, mt // 2, mt % 2, :], in_=vp, func=COPY
                )

            def quad(s, p, h):
                """2-way row-packed score pack: kv pair p (mts 2p, 2p+1),
                n-half h of super s; one 1024-wide exp into the paired fp8
                layout.  Packs double-buffer through ps_sc so exp(q)
                overlaps the score matmuls of pack q+1; consecutive packs
                alternate row-band pairs so their matmuls can overlap."""
                et = etbig[:, s % 2]
                sc_ps = ps_sc.tile([128, 2, NCHUNK], F32, tag="sc", name="scq")
                hsl = slice(s * SUP + h * NCHUNK, s * SUP + (h + 1) * NCHUNK)
                bb = 2 * ((2 * p + h) % 2)   # band pair alternation
                for i in range(2):
                    mt = 2 * p + i
                    band = slice(32 * (bb + i), 32 * (bb + i + 1))
                    nc.tensor.matmul(
                        sc_ps[:, i, :],
                        lhsT=krep_sb[band, mt * 128 : (mt + 1) * 128],
                        rhs=qrep_sb[band, hsl],
                        tile_position=(32 * (bb + i), 0),
                    )
                osl = slice(h * NCHUNK, (h + 1) * NCHUNK)
                nc.scalar.activation(
                    out=et[:, p, :, osl],
                    in_=sc_ps, func=EXP, scale=float(SCALE),
                )

            def make_rs(s):
                """Row-sum state for super s: two DR ones-matmul chains (one
                per n-half) over the 4 kv pairs, emitted in two stages."""
                et = etbig[:, s % 2]
                rs_ps = [
                    ps_rs.tile([1, NCHUNK], F32, tag="rs", name=f"rs{s}_{h}")
                    for h in range(2)
                ]

                def rs_part(h, gs):
                    osl = slice(h * NCHUNK, (h + 1) * NCHUNK)
                    for g in gs:
                        nc.tensor.matmul(
                            rs_ps[h], lhsT=ones8[:, :, 0:1],
                            rhs=et[:, g, :, osl],
                            start=(g == 0), stop=(g == NG - 1), perf_mode=DR,
                        )
                    if gs[-1] == NG - 1:
                        rr = rowp.tile([1, NCHUNK], F16, tag="rs_row")
                        nc.vector.tensor_copy(out=rr, in_=rs_ps[h])
                        return rr

                return rs_part

            # ---------------- pipeline fill (super 0 head) ----------------
            # DMA-paced: after xb chunk cs lands -> q-proj, pooling, k-proj
            # (m quarter cs), super-0 score packs for kv-pair cs, v-proj.
            rs_cur = make_rs(0)
            for cs in range(4):
                qproj(2 * cs); qproj(2 * cs + 1)
                pool(cs)
                kproj(cs)
                quad(0, cs, 0); quad(0, cs, 1)
                vproj(2 * cs); vproj(2 * cs + 1)
                if cs == 2:
                    rs_cur(0, [0, 1])
                if cs == 3:
                    rs_cur(1, [0, 1])

            # ---------------- main loop over n-supers ----------------
            for s in range(NSUP):
                et = etbig[:, s % 2]
                rs_here = rs_cur

                def nquad(p, h):
                    if s + 1 < NSUP:
                        quad(s + 1, p, h)

                outu4 = outup.tile([128, 2, SUP], F8, tag="outu")
                scale_sb = {}

                def agg_chains(c, gs):
                    if gs[0] == 0:
                        agg_ps[c] = {
                            h: ps_av.tile([128, NCHUNK], F32, tag="av",
                                          name=f"av{c}{h}")
                            for h in range(2)
                        }
                    for g in gs:
                        for h in range(2):
                            osl = slice(h * NCHUNK, (h + 1) * NCHUNK)
                            nc.tensor.matmul(
                                agg_ps[c][h],
                                lhsT=vT4_sb[:, g, :, c * 128 : (c + 1) * 128],
                                rhs=et[:, g, :, osl],
                                start=(g == 0), stop=(g == NG - 1),
                                perf_mode=DR,
                            )

                def stt(c):
                    for h in range(2):
                        osl = slice(h * NCHUNK, (h + 1) * NCHUNK)
                        nc.vector.scalar_tensor_tensor(
                            out=outu4[:, c, osl],
                            in0=agg_ps[c][h],
                            scalar=1.0,
                            in1=scale_sb[h],
                            op0=mybir.AluOpType.mult,
                            op1=mybir.AluOpType.mult,
                        )

                agg_ps = {}
                # 1. aggregation c=0; first next-super quads interleave
                agg_chains(0, [0, 1])
                nquad(0, 0); nquad(0, 1)
                if s + 1 >= NSUP:
                    # no next-super packs to cover the row-copy latency:
                    # let the second half of agg c0 do it instead
                    rows = [rs_here(0, [2, 3]), rs_here(1, [2, 3])]
                    agg_chains(0, [2, 3])
                else:
                    agg_chains(0, [2, 3])
                    nquad(1, 0); nquad(1, 1)
                    rows = [rs_here(0, [2, 3]), rs_here(1, [2, 3])]
                # 4. reciprocal broadcast -> per-column scales
                for h in range(2):
                    bc_ps = ps_rs.tile([128, NCHUNK], F32, tag="rs", name="bc")
                    nc.tensor.matmul(bc_ps, lhsT=ones_row, rhs=rows[h])
                    sc_t = sclp.tile([128, NCHUNK], F32, tag="scale")
                    nc.vector.reciprocal_approx_fast(out=sc_t, in_=bc_ps)
                    scale_sb[h] = sc_t
                # rs tiles for s+1 alloc AFTER the bc tiles (ps_rs rotation)
                rs_nxt = make_rs(s + 1) if s + 1 < NSUP else None
                # 5. evacuate c=0; aggregate c=1 with quads interleaved
                stt(0)
                agg_chains(1, [0, 1])
                nquad(2, 0); nquad(2, 1)
                agg_chains(1, [2, 3])
                stt(1)
                if rs_nxt is not None:
                    rs_nxt(0, [0, 1])
                nquad(3, 0); nquad(3, 1)

                # 6. out-projection (fp8 DR) + residual add + store
                for ot in range(2):
                    for half in range(2):
                        osl = slice(half * NCHUNK, (half + 1) * NCHUNK)
                        fsl = slice(s * SUP + half * NCHUNK,
                                    s * SUP + (half + 1) * NCHUNK)
                        op_ps = ps_av.tile([128, NCHUNK], F32, tag="av",
                                           name="op")
                        nc.tensor.matmul(
                            op_ps, lhsT=wo8_sb[:, :, ot, :],
                            rhs=outu4[:, :, osl], perf_mode=DR,
                        )
                        y_st = ypool.tile([128, NCHUNK], F32, tag="y")
                        nc.vector.tensor_add(
                            out=y_st, in0=op_ps, in1=xb_sb[:, ot, fsl]
                        )
                        nc.sync.dma_start(
                            out=y_out[ot * 128 : (ot + 1) * 128, fsl], in_=y_st
                        )
                    if ot == 0:
                        if rs_nxt is not None:
                            rs_nxt(1, [0, 1])
                rs_cur = rs_nxt
    nc.compile()
    return nc


_NC_CACHE = {}


def _get_nc():
    if "nc" not in _NC_CACHE:
        _NC_CACHE["nc"] = build_nc()
    return _NC_CACHE["nc"]


def _prep_inputs(x, wq, wk, wv, wo, gamma):
    bf = ml_dtypes.bfloat16
    x = np.asarray(x, dtype=np.float32)
    xb = x.astype(bf)
    wq4 = np.tile(np.asarray(wq, np.float32).T, (1, 4))
    wk4 = np.tile(np.asarray(wk, np.float32).T * 0.25, (1, 4))
    wvT = np.asarray(wv, np.float32).T * 0.25
    woT = np.float32(np.asarray(gamma, np.float32)[0]) * np.asarray(
        wo, np.float32
    ).T
    wqk = np.ascontiguousarray(
        np.concatenate([wq4, wk4], axis=1)).astype(bf)
    wvo = np.ascontiguousarray(
        np.concatenate([wvT, woT], axis=1)).astype(bf)
    in_maps = []
    for i in range(NCORES):
        in_maps.append({
            "xb": np.ascontiguousarray(xb[i].reshape(C, N)),
            "wqk": wqk,
            "wvo": wvo,
        })
    return in_maps


def run(x, wq, wk, wv, wo, gamma, trace=False, **trace_kwargs):
    nc = _get_nc()
    in_maps = _prep_inputs(x, wq, wk, wv, wo, gamma)
    res = run_bass_kernel_spmd(
        nc, in_maps, list(range(NCORES)), trace=trace, **trace_kwargs
    )
    y = np.stack([res.results[i]["y"].reshape(C, H, W) for i in range(NCORES)])
    return y, res


def kernel(x, wq, wk, wv, wo, gamma):
    y, _ = run(x, wq, wk, wv, wo, gamma, trace=False)
    return y
